# revision 23
# baseline (speedup 1.0000x reference)
"""Trainium2 Bass kernel for CRF mean-field iteration (nn_CRF), v3 (bf16).

Math (derived from the reference):
    comp = -I  =>  each iteration is   x <- x0 + w * smooth(softmax(x, C))
    output = log_softmax(x_final, C)
where smooth = per-channel separable 11-tap Gaussian blur over H then W
('same' zero padding, center tap zeroed, per-sample spacing).

Key reformulation vs v1: x is NEVER materialized in SBUF. Each stage's
W-conv PSUM bank is PRE-SEEDED with x0 (identity matmul, bf16, PE) and the
conv accumulates on top, so PSUM holds x0+s and a single ACT pass gives
m = exp(x0+s) directly. Then S = sum_c m (pairwise tree, Pool),
r = 1/S (DVE fast recip), p = m*r (DVE bf16 4x). Output = Ln(p) [ACT].

Per-stage engine balance (predicted):
  PE   seed 48 + 288 banded bf16 matmuls   ~24 us
  ACT  16 exp (PSUM->bf16) + 16 copies     ~25 us
  DVE  32 copies + mul2 + recip            ~24 us
  Pool channel-sum tree + rb               ~16 us
The PSUM->SBUF o1 copies (the H->W conv handoff) are split DVE:ACT 2:1;
GPSIMD/Pool cannot touch PSUM on this toolchain.

Convs are banded-Toeplitz matmuls with the DATA stationary: out1[w,h'] =
sum_h p[h,w]*Th[h,h'] lands transposed in PSUM; the W-conv transposes
back. Th/Tw built on host from runtime spacing/theta (weight folded into
Tw), shipped bf16.
"""

import sys

if "/opt/trn_rl_repo" not in sys.path:
    sys.path.insert(0, "/opt/trn_rl_repo")

from contextlib import ExitStack

import numpy as np

import concourse.bass as bass
import concourse.tile as tile
from concourse import bacc, mybir

F32 = mybir.dt.float32
BF16 = mybir.dt.bfloat16
AF = mybir.ActivationFunctionType

B, C, H, W = 16, 16, 384, 384
N_CORES = 8
BPC = B // N_CORES  # samples per core
N_ITER = 5
FS = 11
HALF = FS // 2  # 5
P = 128
NCH = H // P  # 3 h-chunks
NCW = W // P  # 3 w-chunks


def _band(j, n):
    """Output-column range touched by contraction chunk j of a banded T."""
    return max(0, P * j - HALF), min(n, P * j + P + HALF)


def _crf_kernel(ctx, tc, out_d, x_in, th_in, tw_in, id_in, n_samples, n_iter):
    nc = tc.nc

    state = ctx.enter_context(tc.tile_pool(name="state", bufs=1))
    xstage = ctx.enter_context(tc.tile_pool(name="xstage", bufs=2))
    o1p = ctx.enter_context(tc.tile_pool(name="o1p", bufs=4))
    mats = ctx.enter_context(tc.tile_pool(name="mats", bufs=2))
    trp = ctx.enter_context(tc.tile_pool(name="trp", bufs=2))
    srp = ctx.enter_context(tc.tile_pool(name="srp", bufs=1))
    outp = ctx.enter_context(tc.tile_pool(name="outp", bufs=2))
    psA = ctx.enter_context(tc.tile_pool(name="psA", bufs=2, space="PSUM"))
    psB = ctx.enter_context(tc.tile_pool(name="psB", bufs=2, space="PSUM"))

    x0b = state.tile([P, C, NCH, W], BF16, tag="x0b")
    mp = state.tile([P, C, NCH, W], BF16, tag="mp")
    ident = state.tile([P, P], BF16, tag="ident")
    nc.sync.dma_start(out=ident[:], in_=id_in[:])

    def emit_tree8(lo):
        """Pairwise channel-sum of mp[:, lo:lo+8] -> [P, 1, NCH, W] bf16.
        Off the critical path: levels split DVE/Pool/DVE."""
        t4 = trp.tile([P, 4, NCH, W], BF16, tag="t4")
        nc.vector.tensor_add(t4[:], mp[:, lo : lo + 8 : 2], mp[:, lo + 1 : lo + 8 : 2])
        t2 = trp.tile([P, 2, NCH, W], BF16, tag="t2")
        nc.gpsimd.tensor_add(t2[:], t4[:, 0:4:2], t4[:, 1:4:2])
        t1 = trp.tile([P, 1, NCH, W], BF16, tag="t1")
        nc.vector.tensor_add(t1[:], t2[:, 0:1], t2[:, 1:2])
        return t1

    def emit_tree4(lo):
        """Channel-sum of mp[:, lo:lo+4] -> [P, 1, NCH, W] bf16 on DVE
        (latency-critical last piece of the stage tail)."""
        t2 = trp.tile([P, 2, NCH, W], BF16, tag="u2")
        nc.vector.tensor_add(t2[:], mp[:, lo : lo + 4 : 2], mp[:, lo + 1 : lo + 4 : 2])
        t1 = trp.tile([P, 1, NCH, W], BF16, tag="u1")
        nc.vector.tensor_add(t1[:], t2[:, 0:1], t2[:, 1:2])
        return t1

    def emit_softmax_tail(tAB1, tB2, ln_out=None, b=None):
        """S=tAB1+tB2; r=1/S; rb=bf16(r); p: mp[:,c] *= rb (in place).
        The tail chain is split by j-chunk so the next stage's H-conv j=0
        matmuls (which only need rb[j=0]) start ~3us earlier. A few mul2
        channels go to the otherwise-idle Pool engine.
        If ln_out, also emit the final Ln+store interleaved per channel pair."""
        S = srp.tile([P, 1, NCH, W], F32, tag="S")
        nc.vector.tensor_add(S[:], tAB1[:], tB2[:])
        r = srp.tile([P, 1, NCH, W], F32, tag="r")
        rb = srp.tile([P, 1, NCH, W], BF16, tag="rb")
        # j-split chain so mul2(0, j=0) (gating the next stage's first
        # H-conv matmul) completes as early as possible
        for j in range(NCH):
            nc.vector.reciprocal_approx_fast(r[:, 0, j], S[:, 0, j])
            nc.vector.tensor_copy(rb[:, 0, j], r[:, 0, j])
            nc.vector.tensor_mul(out=mp[:, 0, j], in0=mp[:, 0, j], in1=rb[:, 0, j])
        for c in range(1, C):
            if 6 <= c < 12 and ln_out is None:
                nc.gpsimd.tensor_mul(out=mp[:, c], in0=mp[:, c], in1=rb[:, 0])
            else:
                nc.vector.tensor_mul(out=mp[:, c], in0=mp[:, c], in1=rb[:, 0])
            if ln_out is not None and c % 2 == 1:
                oq = outp.tile([P, 2, NCH, W], F32, tag="oq")
                nc.scalar.activation(out=oq[:], in_=mp[:, c - 1 : c + 1], func=AF.Ln)
                nc.sync.dma_start(
                    out=ln_out[b, c - 1 : c + 1].rearrange("c (j p) w -> p c j w", p=P),
                    in_=oq[:],
                )

    def emit_seed(pb, c, stop):
        """PSUM <- x0 for channel c via identity matmul (start of group).
        The identity stationary is loaded once; seeds 2 and 3 reuse it."""
        for m in range(NCH):
            nc.tensor.matmul(
                pb[:, m, 0:W],
                lhsT=ident[:],
                rhs=x0b[:, c, m, :],
                start=True,
                stop=stop,
            )

    def emit_loads(b):
        """x DMAs + bf16 casts + Toeplitz DMAs for sample b. Hoisted into
        the PREVIOUS sample's last stage so the boundary hides the DMA."""
        for q in range(4):
            xq = xstage.tile([P, 4, NCH, W], F32, tag="xq")
            nc.sync.dma_start(
                out=xq[:],
                in_=x_in[b, 4 * q : 4 * q + 4].rearrange("c (j p) w -> p c j w", p=P),
            )
            nc.vector.tensor_copy(x0b[:, 4 * q : 4 * q + 4], xq[:])
        th_sb = mats.tile([P, NCH, H], BF16, tag="th")
        tw_sb = mats.tile([P, NCW, W], BF16, tag="tw")
        nc.sync.dma_start(out=th_sb[:], in_=th_in[b].rearrange("(j p) n -> p j n", p=P))
        nc.sync.dma_start(out=tw_sb[:], in_=tw_in[b].rearrange("(j p) n -> p j n", p=P))
        return th_sb, tw_sb

    loads = emit_loads(0)
    for b in range(n_samples):
        th_sb, tw_sb = loads

        # ---- stage 0: m = exp(x0) straight from SBUF; p0 = m / sum_c m ----
        for q in range(4):
            nc.scalar.activation(
                out=mp[:, 4 * q : 4 * q + 4], in_=x0b[:, 4 * q : 4 * q + 4], func=AF.Exp
            )
            if q == 1:
                tA = emit_tree8(0)
            elif q == 2:
                tB1 = emit_tree4(8)
                tAB1 = srp.tile([P, 1, NCH, W], BF16, tag="tAB1")
                nc.gpsimd.tensor_add(tAB1[:], tA[:], tB1[:])
            elif q == 3:
                tB2 = emit_tree4(12)
        emit_softmax_tail(tAB1, tB2)

        # ---- stages 1..n_iter: psum = x0 + conv(p); m = exp(psum); p = m/S ----
        # Software-pipelined: H-conv(c) is emitted two channels ahead of
        # seed/W-conv/exp(c) so the PE never waits on the PSUM->SBUF
        # handoff copies and exp gets two channels of slack.
        LAG = 3
        preseeded = {}
        for it in range(n_iter):
            last = it == n_iter - 1
            tA = tAB1 = tB2 = None
            o1ts = {}
            for c in range(C + LAG):
                # W-conv/exp of the lagged channel go FIRST within the block:
                # their deps are LAG blocks old, so ACT/DVE never arrive at an
                # instruction whose producer was just issued.
                if c >= LAG:
                    cc = c - LAG
                    o1t = o1ts.pop(cc)
                    # W-conv on top of the x0 seed (possibly pre-seeded
                    # during the previous stage's tail barrier)
                    pb = preseeded.pop(cc, None)
                    if pb is None:
                        pb = psB.tile([P, NCH, 512], F32, tag="pb")
                        emit_seed(pb, cc, stop=False)
                    for m in range(NCH):
                        for j in range(NCW):
                            n0, n1 = _band(j, W)
                            nc.tensor.matmul(
                                pb[:, m, n0:n1],
                                lhsT=o1t[:, j, m * P : (m + 1) * P],
                                rhs=tw_sb[:, j, n0:n1],
                                start=False,
                                stop=(j == NCW - 1),
                            )
                    nc.scalar.activation(out=mp[:, cc], in_=pb[:, :, 0:W], func=AF.Exp)
                    if cc == 7:
                        tA = emit_tree8(0)
                    elif cc == 11:
                        tB1 = emit_tree4(8)
                        tAB1 = srp.tile([P, 1, NCH, W], BF16, tag="tAB1")
                        nc.gpsimd.tensor_add(tAB1[:], tA[:], tB1[:])
                    elif cc == 15:
                        tB2 = emit_tree4(12)
                if c < C:
                    # H-conv: out1[w, h'] = sum_h p[h, w] * Th[h, h']
                    o1t = o1p.tile([P, NCW, H], BF16, tag="o1")
                    o1ts[c] = o1t
                    for m in range(NCW):
                        pa = psA.tile([P, 1, 512], F32, tag="pa")
                        for j in range(NCH):
                            n0, n1 = _band(j, H)
                            nc.tensor.matmul(
                                pa[:, 0, n0:n1],
                                lhsT=mp[:, c, j, m * P : (m + 1) * P],
                                rhs=th_sb[:, j, n0:n1],
                                start=(j == 0),
                                stop=(j == NCH - 1),
                            )
                        # PSUM->SBUF handoff: split ACT/DVE by channel parity
                        if (c + m) % 2 == 0:
                            nc.scalar.copy(out=o1t[:, m], in_=pa[:, 0, 0:H])
                        else:
                            nc.vector.tensor_copy(o1t[:, m], pa[:, 0, 0:H])
            if last and b + 1 < n_samples:
                loads = emit_loads(b + 1)
            if not last:
                # fill the softmax-tail barrier with next-stage seeds (they
                # depend only on x0, keeping the PE busy through the tail)
                for cn in range(2):
                    pbn = psB.tile([P, NCH, 512], F32, tag="pb")
                    emit_seed(pbn, cn, stop=False)
                    preseeded[cn] = pbn
            emit_softmax_tail(tAB1, tB2, ln_out=out_d if last else None, b=b)


def build_nc(n_samples=BPC, n_iter=N_ITER, full_j0=False):
    # Bacc (not plain Bass): its compile() pass legalizes multi-wait
    # instructions via InstEventSemaphore — walrus caps regular instructions
    # at ONE sync wait.
    nc = bacc.Bacc()
    x_in = nc.dram_tensor("x", [n_samples, C, H, W], F32, kind="ExternalInput")
    th_in = nc.dram_tensor("th", [n_samples, H, H], BF16, kind="ExternalInput")
    tw_in = nc.dram_tensor("tw", [n_samples, W, W], BF16, kind="ExternalInput")
    id_in = nc.dram_tensor("ident", [P, P], BF16, kind="ExternalInput")
    out_d = nc.dram_tensor("out", [n_samples, C, H, W], F32, kind="ExternalOutput")
    with tile.TileContext(nc) as tc:
        with ExitStack() as ctx:
            _crf_kernel(ctx, tc, out_d, x_in, th_in, tw_in, id_in, n_samples, n_iter)
    nc.finalize()
    return nc


def make_toeplitz(spacing, inv_theta, size, weight=1.0):
    """Banded symmetric Toeplitz matrix for the 1D 'same' correlation."""
    d = spacing * np.arange(-(FS // 2), FS // 2 + 1, dtype=np.float32)
    k = np.exp(-((d * inv_theta) ** 2) / 2.0).astype(np.float32)
    k[FS // 2] = 0.0
    t = np.zeros((size, size), dtype=np.float32)
    for tap in range(FS):
        off = tap - FS // 2  # out[h] += k[tap] * x[h + off]
        idx = np.arange(max(0, -off), min(size, size - off))
        t[idx + off, idx] = k[tap]
    return (t * weight).astype(np.float32)


def host_prep(x, spatial_spacings, smoothness_weight, inv_smoothness_theta):
    """Build per-sample Th (H-conv) and weight-scaled Tw (W-conv), bf16."""
    import ml_dtypes

    w = float(np.asarray(smoothness_weight))
    th = np.stack(
        [
            make_toeplitz(float(spatial_spacings[b, 0]), float(inv_smoothness_theta[0]), H)
            for b in range(x.shape[0])
        ]
    ).astype(ml_dtypes.bfloat16)
    tw = np.stack(
        [
            make_toeplitz(
                float(spatial_spacings[b, 1]), float(inv_smoothness_theta[1]), W, weight=w
            )
            for b in range(x.shape[0])
        ]
    ).astype(ml_dtypes.bfloat16)
    return th, tw


def make_ident():
    import ml_dtypes

    return np.eye(P, dtype=np.float32).astype(ml_dtypes.bfloat16)


_NC_CACHE = {}


def kernel(x, spatial_spacings, smoothness_weight, inv_smoothness_theta):
    from concourse.bass_utils import run_bass_kernel_spmd

    x = np.ascontiguousarray(np.asarray(x), dtype=np.float32)
    spatial_spacings = np.asarray(spatial_spacings, dtype=np.float32)
    th, tw = host_prep(x, spatial_spacings, smoothness_weight, inv_smoothness_theta)
    ident = make_ident()

    key = (BPC, N_ITER)
    if key not in _NC_CACHE:
        _NC_CACHE[key] = build_nc(BPC, N_ITER)
    nc = _NC_CACHE[key]

    core_ids = list(range(N_CORES))
    in_maps = []
    for i in core_ids:
        sl = slice(i * BPC, (i + 1) * BPC)
        in_maps.append({"x": x[sl], "th": th[sl], "tw": tw[sl], "ident": ident})
    res = run_bass_kernel_spmd(nc, in_maps, core_ids)
    out = np.concatenate([res.results[i]["out"] for i in core_ids], axis=0)
    return out.astype(np.float32)


if __name__ == "__main__":
    rng = np.random.default_rng(0)
    x = rng.standard_normal((B, C, H, W), dtype=np.float32)
    out = kernel(
        x,
        np.ones((B, 2), np.float32),
        np.float32(1.0),
        np.ones((2,), np.float32),
    )
    print(out.shape, out.dtype)


# revision 24
# speedup vs baseline: 1.1519x; 1.1519x over previous
"""Trainium2 Bass kernel for CRF mean-field iteration (nn_CRF), v3 (bf16).

Math (derived from the reference):
    comp = -I  =>  each iteration is   x <- x0 + w * smooth(softmax(x, C))
    output = log_softmax(x_final, C)
where smooth = per-channel separable 11-tap Gaussian blur over H then W
('same' zero padding, center tap zeroed, per-sample spacing).

Key reformulation vs v1: x is NEVER materialized in SBUF. Each stage's
W-conv PSUM bank is PRE-SEEDED with x0 (identity matmul, bf16, PE) and the
conv accumulates on top, so PSUM holds x0+s and a single ACT pass gives
m = exp(x0+s) directly. Then S = sum_c m (pairwise tree, Pool),
r = 1/S (DVE fast recip), p = m*r (DVE bf16 4x). Output = Ln(p) [ACT].

Per-stage engine balance (predicted):
  PE   seed 48 + 288 banded bf16 matmuls   ~24 us
  ACT  16 exp (PSUM->bf16) + 16 copies     ~25 us
  DVE  32 copies + mul2 + recip            ~24 us
  Pool channel-sum tree + rb               ~16 us
The PSUM->SBUF o1 copies (the H->W conv handoff) are split DVE:ACT 2:1;
GPSIMD/Pool cannot touch PSUM on this toolchain.

Convs are banded-Toeplitz matmuls with the DATA stationary: out1[w,h'] =
sum_h p[h,w]*Th[h,h'] lands transposed in PSUM; the W-conv transposes
back. Th/Tw built on host from runtime spacing/theta (weight folded into
Tw), shipped bf16.
"""

import sys

if "/opt/trn_rl_repo" not in sys.path:
    sys.path.insert(0, "/opt/trn_rl_repo")

from contextlib import ExitStack

import numpy as np

import concourse.bass as bass
import concourse.tile as tile
from concourse import bacc, mybir

F32 = mybir.dt.float32
BF16 = mybir.dt.bfloat16
AF = mybir.ActivationFunctionType

B, C, H, W = 16, 16, 384, 384
N_CORES = 8
BPC = B // N_CORES  # samples per core
N_ITER = 5
FS = 11
HALF = FS // 2  # 5
P = 128
NCH = H // P  # 3 h-chunks
NCW = W // P  # 3 w-chunks


def _band(j, n):
    """Output-column range touched by contraction chunk j of a banded T."""
    return max(0, P * j - HALF), min(n, P * j + P + HALF)


def _crf_kernel(ctx, tc, out_d, x_in, th_in, tw_in, id_in, n_samples, n_iter):
    nc = tc.nc

    state = ctx.enter_context(tc.tile_pool(name="state", bufs=1))
    xstage = ctx.enter_context(tc.tile_pool(name="xstage", bufs=2))
    o1p = ctx.enter_context(tc.tile_pool(name="o1p", bufs=4))
    mats = ctx.enter_context(tc.tile_pool(name="mats", bufs=2))
    trp = ctx.enter_context(tc.tile_pool(name="trp", bufs=2))
    srp = ctx.enter_context(tc.tile_pool(name="srp", bufs=1))
    outp = ctx.enter_context(tc.tile_pool(name="outp", bufs=2))
    psA = ctx.enter_context(tc.tile_pool(name="psA", bufs=2, space="PSUM"))
    psB = ctx.enter_context(tc.tile_pool(name="psB", bufs=2, space="PSUM"))

    x0b = state.tile([P, C, NCH, W], BF16, tag="x0b")
    mp = state.tile([P, C, NCH, W], BF16, tag="mp")
    ident = state.tile([P, P], BF16, tag="ident")
    nc.sync.dma_start(out=ident[:], in_=id_in[:])

    def emit_tree8(lo):
        """Pairwise channel-sum of mp[:, lo:lo+8] -> [P, 1, NCH, W] bf16.
        Off the critical path: levels split DVE/Pool/DVE."""
        t4 = trp.tile([P, 4, NCH, W], BF16, tag="t4")
        nc.vector.tensor_add(t4[:], mp[:, lo : lo + 8 : 2], mp[:, lo + 1 : lo + 8 : 2])
        t2 = trp.tile([P, 2, NCH, W], BF16, tag="t2")
        nc.gpsimd.tensor_add(t2[:], t4[:, 0:4:2], t4[:, 1:4:2])
        t1 = trp.tile([P, 1, NCH, W], BF16, tag="t1")
        nc.vector.tensor_add(t1[:], t2[:, 0:1], t2[:, 1:2])
        return t1

    def emit_tree4(lo):
        """Channel-sum of mp[:, lo:lo+4] -> [P, 1, NCH, W] bf16 on DVE
        (latency-critical last piece of the stage tail)."""
        t2 = trp.tile([P, 2, NCH, W], BF16, tag="u2")
        nc.vector.tensor_add(t2[:], mp[:, lo : lo + 4 : 2], mp[:, lo + 1 : lo + 4 : 2])
        t1 = trp.tile([P, 1, NCH, W], BF16, tag="u1")
        nc.vector.tensor_add(t1[:], t2[:, 0:1], t2[:, 1:2])
        return t1

    def emit_softmax_tail(tAB1, tB2, ln_out=None, b=None):
        """S=tAB1+tB2; r=1/S; rb=bf16(r); p: mp[:,c] *= rb (in place).
        The tail chain is split by j-chunk so the next stage's H-conv j=0
        matmuls (which only need rb[j=0]) start ~3us earlier. A few mul2
        channels go to the otherwise-idle Pool engine.
        If ln_out, also emit the final Ln+store interleaved per channel pair."""
        S = srp.tile([P, 1, NCH, W], F32, tag="S")
        nc.vector.tensor_add(S[:], tAB1[:], tB2[:])
        r = srp.tile([P, 1, NCH, W], F32, tag="r")
        nc.vector.reciprocal_approx_fast(r[:, 0], S[:, 0])
        rb = srp.tile([P, 1, NCH, W], BF16, tag="rb")
        nc.vector.tensor_copy(rb[:], r[:])
        for c in range(C):
            nc.vector.tensor_mul(out=mp[:, c], in0=mp[:, c], in1=rb[:, 0])
            if ln_out is not None and c % 2 == 1:
                oq = outp.tile([P, 2, NCH, W], F32, tag="oq")
                nc.scalar.activation(out=oq[:], in_=mp[:, c - 1 : c + 1], func=AF.Ln)
                nc.sync.dma_start(
                    out=ln_out[b, c - 1 : c + 1].rearrange("c (j p) w -> p c j w", p=P),
                    in_=oq[:],
                )

    def emit_seed(pb, c, stop):
        """PSUM <- x0 for channel c via identity matmul (start of group).
        The identity stationary is loaded once; seeds 2 and 3 reuse it."""
        for m in range(NCH):
            nc.tensor.matmul(
                pb[:, m, 0:W],
                lhsT=ident[:],
                rhs=x0b[:, c, m, :],
                start=True,
                stop=stop,
            )

    def emit_loads(b):
        """x DMAs + bf16 casts + Toeplitz DMAs for sample b. Hoisted into
        the PREVIOUS sample's last stage so the boundary hides the DMA."""
        for q in range(4):
            xq = xstage.tile([P, 4, NCH, W], F32, tag="xq")
            nc.sync.dma_start(
                out=xq[:],
                in_=x_in[b, 4 * q : 4 * q + 4].rearrange("c (j p) w -> p c j w", p=P),
            )
            nc.vector.tensor_copy(x0b[:, 4 * q : 4 * q + 4], xq[:])
        th_sb = mats.tile([P, NCH, H], BF16, tag="th")
        tw_sb = mats.tile([P, NCW, W], BF16, tag="tw")
        nc.sync.dma_start(out=th_sb[:], in_=th_in[b].rearrange("(j p) n -> p j n", p=P))
        nc.sync.dma_start(out=tw_sb[:], in_=tw_in[b].rearrange("(j p) n -> p j n", p=P))
        return th_sb, tw_sb

    loads = emit_loads(0)
    for b in range(n_samples):
        th_sb, tw_sb = loads

        # ---- stage 0: m = exp(x0) straight from SBUF; p0 = m / sum_c m ----
        for q in range(4):
            nc.scalar.activation(
                out=mp[:, 4 * q : 4 * q + 4], in_=x0b[:, 4 * q : 4 * q + 4], func=AF.Exp
            )
            if q == 1:
                tA = emit_tree8(0)
            elif q == 2:
                tB1 = emit_tree4(8)
                tAB1 = srp.tile([P, 1, NCH, W], BF16, tag="tAB1")
                nc.gpsimd.tensor_add(tAB1[:], tA[:], tB1[:])
            elif q == 3:
                tB2 = emit_tree4(12)
        emit_softmax_tail(tAB1, tB2)

        # ---- stages 1..n_iter: psum = x0 + conv(p); m = exp(psum); p = m/S ----
        # Software-pipelined: H-conv(c) is emitted two channels ahead of
        # seed/W-conv/exp(c) so the PE never waits on the PSUM->SBUF
        # handoff copies and exp gets two channels of slack.
        LAG = 3
        preseeded = {}
        for it in range(n_iter):
            last = it == n_iter - 1
            tA = tAB1 = tB2 = None
            o1ts = {}
            for c in range(C + LAG):
                # W-conv/exp of the lagged channel go FIRST within the block:
                # their deps are LAG blocks old, so ACT/DVE never arrive at an
                # instruction whose producer was just issued.
                if c >= LAG:
                    cc = c - LAG
                    o1t = o1ts.pop(cc)
                    # W-conv on top of the x0 seed (possibly pre-seeded
                    # during the previous stage's tail barrier)
                    pb = preseeded.pop(cc, None)
                    if pb is None:
                        pb = psB.tile([P, NCH, 512], F32, tag="pb")
                        emit_seed(pb, cc, stop=False)
                    for m in range(NCH):
                        for j in range(NCW):
                            n0, n1 = _band(j, W)
                            nc.tensor.matmul(
                                pb[:, m, n0:n1],
                                lhsT=o1t[:, j, m * P : (m + 1) * P],
                                rhs=tw_sb[:, j, n0:n1],
                                start=False,
                                stop=(j == NCW - 1),
                            )
                    nc.scalar.activation(out=mp[:, cc], in_=pb[:, :, 0:W], func=AF.Exp)
                    if cc == 7:
                        tA = emit_tree8(0)
                    elif cc == 11:
                        tB1 = emit_tree4(8)
                        tAB1 = srp.tile([P, 1, NCH, W], BF16, tag="tAB1")
                        nc.gpsimd.tensor_add(tAB1[:], tA[:], tB1[:])
                    elif cc == 15:
                        tB2 = emit_tree4(12)
                if c < C:
                    # H-conv: out1[w, h'] = sum_h p[h, w] * Th[h, h']
                    o1t = o1p.tile([P, NCW, H], BF16, tag="o1")
                    o1ts[c] = o1t
                    for m in range(NCW):
                        pa = psA.tile([P, 1, 512], F32, tag="pa")
                        for j in range(NCH):
                            n0, n1 = _band(j, H)
                            nc.tensor.matmul(
                                pa[:, 0, n0:n1],
                                lhsT=mp[:, c, j, m * P : (m + 1) * P],
                                rhs=th_sb[:, j, n0:n1],
                                start=(j == 0),
                                stop=(j == NCH - 1),
                            )
                        # PSUM->SBUF handoff: split ACT/DVE by channel parity
                        if (c + m) % 2 == 0:
                            nc.scalar.copy(out=o1t[:, m], in_=pa[:, 0, 0:H])
                        else:
                            nc.vector.tensor_copy(o1t[:, m], pa[:, 0, 0:H])
            if last and b + 1 < n_samples:
                loads = emit_loads(b + 1)
            if not last:
                # fill the softmax-tail barrier with next-stage seeds (they
                # depend only on x0, keeping the PE busy through the tail)
                for cn in range(2):
                    pbn = psB.tile([P, NCH, 512], F32, tag="pb")
                    emit_seed(pbn, cn, stop=False)
                    preseeded[cn] = pbn
            emit_softmax_tail(tAB1, tB2, ln_out=out_d if last else None, b=b)


def build_nc(n_samples=BPC, n_iter=N_ITER, full_j0=False):
    # Bacc (not plain Bass): its compile() pass legalizes multi-wait
    # instructions via InstEventSemaphore — walrus caps regular instructions
    # at ONE sync wait.
    nc = bacc.Bacc()
    x_in = nc.dram_tensor("x", [n_samples, C, H, W], F32, kind="ExternalInput")
    th_in = nc.dram_tensor("th", [n_samples, H, H], BF16, kind="ExternalInput")
    tw_in = nc.dram_tensor("tw", [n_samples, W, W], BF16, kind="ExternalInput")
    id_in = nc.dram_tensor("ident", [P, P], BF16, kind="ExternalInput")
    out_d = nc.dram_tensor("out", [n_samples, C, H, W], F32, kind="ExternalOutput")
    with tile.TileContext(nc) as tc:
        with ExitStack() as ctx:
            _crf_kernel(ctx, tc, out_d, x_in, th_in, tw_in, id_in, n_samples, n_iter)
    nc.finalize()
    return nc


def make_toeplitz(spacing, inv_theta, size, weight=1.0):
    """Banded symmetric Toeplitz matrix for the 1D 'same' correlation."""
    d = spacing * np.arange(-(FS // 2), FS // 2 + 1, dtype=np.float32)
    k = np.exp(-((d * inv_theta) ** 2) / 2.0).astype(np.float32)
    k[FS // 2] = 0.0
    t = np.zeros((size, size), dtype=np.float32)
    for tap in range(FS):
        off = tap - FS // 2  # out[h] += k[tap] * x[h + off]
        idx = np.arange(max(0, -off), min(size, size - off))
        t[idx + off, idx] = k[tap]
    return (t * weight).astype(np.float32)


def host_prep(x, spatial_spacings, smoothness_weight, inv_smoothness_theta):
    """Build per-sample Th (H-conv) and weight-scaled Tw (W-conv), bf16."""
    import ml_dtypes

    w = float(np.asarray(smoothness_weight))
    th = np.stack(
        [
            make_toeplitz(float(spatial_spacings[b, 0]), float(inv_smoothness_theta[0]), H)
            for b in range(x.shape[0])
        ]
    ).astype(ml_dtypes.bfloat16)
    tw = np.stack(
        [
            make_toeplitz(
                float(spatial_spacings[b, 1]), float(inv_smoothness_theta[1]), W, weight=w
            )
            for b in range(x.shape[0])
        ]
    ).astype(ml_dtypes.bfloat16)
    return th, tw


def make_ident():
    import ml_dtypes

    return np.eye(P, dtype=np.float32).astype(ml_dtypes.bfloat16)


_NC_CACHE = {}


def kernel(x, spatial_spacings, smoothness_weight, inv_smoothness_theta):
    from concourse.bass_utils import run_bass_kernel_spmd

    x = np.ascontiguousarray(np.asarray(x), dtype=np.float32)
    spatial_spacings = np.asarray(spatial_spacings, dtype=np.float32)
    th, tw = host_prep(x, spatial_spacings, smoothness_weight, inv_smoothness_theta)
    ident = make_ident()

    key = (BPC, N_ITER)
    if key not in _NC_CACHE:
        _NC_CACHE[key] = build_nc(BPC, N_ITER)
    nc = _NC_CACHE[key]

    core_ids = list(range(N_CORES))
    in_maps = []
    for i in core_ids:
        sl = slice(i * BPC, (i + 1) * BPC)
        in_maps.append({"x": x[sl], "th": th[sl], "tw": tw[sl], "ident": ident})
    res = run_bass_kernel_spmd(nc, in_maps, core_ids)
    out = np.concatenate([res.results[i]["out"] for i in core_ids], axis=0)
    return out.astype(np.float32)


if __name__ == "__main__":
    rng = np.random.default_rng(0)
    x = rng.standard_normal((B, C, H, W), dtype=np.float32)
    out = kernel(
        x,
        np.ones((B, 2), np.float32),
        np.float32(1.0),
        np.ones((2,), np.float32),
    )
    print(out.shape, out.dtype)
